# revision 19
# baseline (speedup 1.0000x reference)
"""Trainium2 Bass kernel for nn_MI_35115652612725 (mutual-information loss).

Math (see reference): per h-slice,
  xs = softmax(x_seen[.,h]/T, -1)  (h, N, C1),  xu = softmax(x_unseen/T, -1)^T
  p_joint = xu @ xs / N;  p_seen/p_unseen are its marginals.
  out = mean_h[ -sum p_joint*(log p_joint - log p_seen) + sum p_unseen*log p_unseen ]

Sharding: 8 cores = (h=4) x (C2-half=2).  Core 2h+half loads x_seen[:,h]
(both v slabs, full C1) and x_unseen[:,h,:,half] (both v, 1024 of 2048
columns) and computes p_joint^T[k, c-half] with the FULL N=4096 contraction,
so no p_joint reduction collective is needed.  The xu softmax normalizer
su[n] spans both column halves, so per-row partial sums are AllReduce'd over
core pairs in eight 2KB chunks, pipelined behind the input stream.

The GEMM runs in fp8e4 DoubleRow (2 contraction subtiles per matmul).  Both
operands are true softmaxes scaled by 128 into fp8 range (trn2 float8e4 is
IEEE e4m3: max finite 240 — a 256 scale overflows top-1 softmax entries to
inf).  p_joint = psum / (N*128*128).  Entries fp8 flushes to zero sit far
below the EPS=1e-7 clamp.  PSUM accumulates pair-groups of 4 (1024 rows) per
bank; groups are added into f32 SBUF accumulators so the PE never waits for
a full-contraction residency.

p_seen = column sums of xsw (= 128*softmax_s, over the full local N) via
all-ones DoubleRow matmuls, p_unseen = column sums of eu_q likewise — both
local, no tail collective.  log(p_seen) is computed on the [1,1024] layout
and redistributed to the [128 x kb] layout with 8 tiny DMAs.  Each core
emits [s1, s2] partials; host computes (sum s2 - sum s1)/4.
"""

import numpy as np

import concourse.bass as bass
import concourse.bacc as bacc
import concourse.mybir as mybir
from concourse import tile
from concourse.bass_utils import run_bass_kernel_spmd

F32 = mybir.dt.float32
BF16 = mybir.dt.bfloat16
FP8 = mybir.dt.float8e4
AF = mybir.ActivationFunctionType
ALU = mybir.AluOpType
AX = mybir.AxisListType
DR = mybir.MatmulPerfMode.DoubleRow

V, H, B, C1, C2 = 2, 4, 2048, 1024, 2048
N = V * B              # 4096 contraction rows per h
P = 128
NP = 16                # pair tiles (256 rows each)
KB = C1 // P           # 8 stationary k-blocks
CHW = 512              # psum chunk width (c columns)
CH = 2                 # chunks per c-half (1024 local columns)
NG = 4                 # pair groups (psum sweep segments)
GP = NP // NG          # pairs per sweep group
SCALE = 20.0           # 1/TEMP
SHIFT = -90.0
EPS = 1e-7
SU8 = 128.0            # eu_q = SU8 * softmax_u  (IEEE e4m3 max 240)
SS8 = 128.0            # xsw = SS8 * softmax_s
ZPJ = 1.0 / (N * SU8 * SS8)   # p_joint = psum * ZPJ
EPSQ = EPS / ZPJ              # EPS in psum scale (clamp before rescale)
LNZ = float(np.log(ZPJ))      # ln(ZPJ): p*ln(p) = ZPJ*(q*ln(q) + ln(ZPJ)*q)
ZPU = 1.0 / (N * SU8)         # p_unseen / p_seen = ones-psum * ZPU
PAIRS = [[0, 1], [2, 3], [4, 5], [6, 7]]

_NC = None
LAST_RESULTS = None


def _build_nc(debug=False):
    nc = bacc.Bacc(None, num_devices=8)
    # Pre-register the exp shift const AP (memset + barrier before any tile
    # instruction) so Exp activations don't pick up an extra sync-wait.
    shift_t = nc.alloc_sbuf_tensor(f"const-float32-{SHIFT}", [128, 1], F32)
    nc.gpsimd.memset(shift_t.ap(), SHIFT)
    nc.const_aps.aps[(F32, SHIFT)] = shift_t.ap()
    nc.all_engine_barrier()

    xu_d = [nc.dram_tensor(f"xu{v}", [B, C1], F32, kind="ExternalInput")
            for v in range(V)]
    xs_d = [nc.dram_tensor(f"xs{v}", [B, C1], F32, kind="ExternalInput")
            for v in range(V)]
    out_d = nc.dram_tensor("parts", [1, 2], F32, kind="ExternalOutput")

    # pair-tile dram views: rows 256j+128i+p -> [j][p, i, c]
    xu_pairs, xs_pairs = [], []
    for v in range(V):
        xuv = xu_d[v][:].rearrange("(t i p) c -> t p i c", p=P, i=2)
        xsv = xs_d[v][:].rearrange("(t i p) c -> t p i c", p=P, i=2)
        for t in range(NP // V):
            xu_pairs.append(xuv[t])
            xs_pairs.append(xsv[t])

    with tile.TileContext(nc) as tc:
        with (
            tc.tile_pool(name="dram", bufs=1, space="DRAM") as dram,
            tc.tile_pool(name="xu_raw", bufs=2) as pool_xur,
            tc.tile_pool(name="xs_raw", bufs=2) as pool_xsr,
            tc.tile_pool(name="xu_e", bufs=10) as pool_xue,
            tc.tile_pool(name="xs_e", bufs=2) as pool_xse,
            tc.tile_pool(name="euq", bufs=NP) as pool_euq,
            tc.tile_pool(name="xsw", bufs=NP) as pool_xsw,
            tc.tile_pool(name="pja", bufs=KB) as pool_pja,
            tc.tile_pool(name="dchain", bufs=2) as pool_d,
            tc.tile_pool(name="vec1", bufs=1) as pool_v1,
            tc.tile_pool(name="stat", bufs=1) as stat,
            tc.tile_pool(name="psum", bufs=3, space="PSUM") as psum,
            tc.tile_pool(name="psum_pu", bufs=2, space="PSUM") as psum_pu,
            tc.tile_pool(name="psum_ps", bufs=2, space="PSUM") as psum_ps,
            tc.tile_pool(name="psum_fin", bufs=1, space="PSUM") as psum_fin,
        ):
            NSU = 2  # su exchange chunks (8 pairs = 2048 rows each)
            SUW = 2 * NP // NSU  # su columns per chunk (16)
            su_in = [dram.tile([P * SUW], F32, name=f"su_in{q}")
                     for q in range(NSU)]
            su_out = [dram.tile([P * SUW], F32, name=f"su_out{q}")
                      for q in range(NSU)]
            lps_dram = dram.tile([C1], F32, name="lps_dram")

            # persistent stats
            su_all = stat.tile([P, 2 * NP], F32)   # local xu row sums
            su_tot = stat.tile([P, 2 * NP], F32)   # after AllReduce
            rsu = stat.tile([P, 2 * NP], F32)
            ss_all = stat.tile([P, 2 * NP], F32)
            rss = stat.tile([P, 2 * NP], F32)
            b_clm = stat.tile([P, KB], F32)        # clamped local rowsums
            a_col = stat.tile([P, KB], F32)        # sum_c p log p per k
            ones_f = stat.tile([P, 1], F32)
            nc.vector.memset(ones_f[:], 1.0)
            ones2_f = stat.tile([P, 2, 16], F32)
            nc.vector.memset(ones2_f[:], 1.0)
            ones8_t = stat.tile([P, 2, 16], FP8)
            nc.vector.tensor_copy(ones8_t[:], ones2_f[:])
            ones8 = ones8_t[:, :, 0:1]

            euq = [pool_euq.tile([P, 2, C1], FP8, name=f"euq{j}", tag="euq")
                   for j in range(NP)]
            xsw = [pool_xsw.tile([P, 2, C1], FP8, name=f"xsw{j}", tag="xsw")
                   for j in range(NP)]
            pja = [pool_pja.tile([P, C1], F32, name=f"pja{k}", tag="pja")
                   for k in range(KB)]
            pu_ps = [psum_pu.tile([1, CHW], F32, tag="pu", name=f"pu{ch}")
                     for ch in range(CH)]
            ps_ps = [psum_ps.tile([1, CHW], F32, tag="ps", name=f"ps{ch}")
                     for ch in range(CH)]

            xue_tiles = [None] * NP

            def emit_xu(j):
                xur = pool_xur.tile([P, 2, C1], F32, name=f"xur{j}", tag="xur")
                nc.sync.dma_start(xur[:], xu_pairs[j])
                xue = pool_xue.tile([P, 2, C1], BF16, name=f"xue{j}", tag="xue")
                for i in range(2):
                    nc.scalar.activation(
                        xue[:, i, :], xur[:, i, :], AF.Exp,
                        bias=SHIFT, scale=SCALE,
                        accum_out=su_all[:, 2 * j + i : 2 * j + i + 1],
                    )
                xue_tiles[j] = xue

            def emit_xs(j):
                xsr = pool_xsr.tile([P, 2, C1], F32, name=f"xsr{j}", tag="xsr")
                nc.sync.dma_start(xsr[:], xs_pairs[j])
                xse = pool_xse.tile([P, 2, C1], BF16, name=f"xse{j}", tag="xse")
                for i in range(2):
                    nc.scalar.activation(
                        xse[:, i, :], xsr[:, i, :], AF.Exp,
                        bias=SHIFT, scale=SCALE,
                        accum_out=ss_all[:, 2 * j + i : 2 * j + i + 1],
                    )
                nc.vector.reciprocal(
                    rss[:, 2 * j : 2 * j + 2], ss_all[:, 2 * j : 2 * j + 2]
                )
                for i in range(2):
                    nc.gpsimd.tensor_scalar(
                        xsw[j][:, i, :], xse[:, i, :],
                        rss[:, 2 * j + i : 2 * j + i + 1], SS8,
                        op0=ALU.mult, op1=ALU.mult,
                    )

            def emit_su_chain(q):
                # pairs 8q..8q+7 row sums: AllReduce with peer half
                cols = slice(SUW * q, SUW * (q + 1))
                view_in = su_in[q][:].rearrange("(p c) -> p c", c=SUW)
                view_out = su_out[q][:].rearrange("(p c) -> p c", c=SUW)
                nc.sync.dma_start(view_in, su_all[:, cols])
                nc.gpsimd.collective_compute(
                    "AllReduce", ALU.add, replica_groups=PAIRS,
                    ins=[su_in[q][:].opt()], outs=[su_out[q][:].opt()],
                )
                nc.sync.dma_start(su_tot[:, cols], view_out)
                nc.vector.reciprocal(rsu[:, cols], su_tot[:, cols])

            def emit_eu_apply(j):
                for i in range(2):
                    nc.gpsimd.tensor_scalar(
                        euq[j][:, i, :], xue_tiles[j][:, i, :],
                        rsu[:, 2 * j + i : 2 * j + i + 1], SU8,
                        op0=ALU.mult, op1=ALU.mult,
                    )

            def emit_sweep(g):
                j0 = GP * g
                for kb in range(KB):
                    for ch in range(CH):
                        pt = psum.tile([P, CHW], F32, tag="mm",
                                       name=f"mm{g}_{kb}_{ch}")
                        for j in range(j0, j0 + GP):
                            nc.tensor.matmul(
                                pt[:],
                                xsw[j][:, :, kb * P : (kb + 1) * P],
                                euq[j][:, :, ch * CHW : (ch + 1) * CHW],
                                start=(j == j0), stop=(j == j0 + GP - 1),
                                perf_mode=DR,
                            )
                        dst = pja[kb][:, ch * CHW : (ch + 1) * CHW]
                        if g == 0:
                            nc.vector.tensor_copy(dst, pt[:])
                        else:
                            nc.vector.tensor_tensor(dst, dst, pt[:], op=ALU.add)
                # p_unseen / p_seen partials: ones^T @ {eu_q, xsw}
                for j in range(j0, j0 + GP):
                    for ch in range(CH):
                        nc.tensor.matmul(
                            pu_ps[ch][:], ones8,
                            euq[j][:, :, ch * CHW : (ch + 1) * CHW],
                            start=(j == 0), stop=(j == NP - 1),
                            perf_mode=DR, skip_group_check=True,
                        )
                    for ch in range(CH):
                        nc.tensor.matmul(
                            ps_ps[ch][:], ones8,
                            xsw[j][:, :, ch * CHW : (ch + 1) * CHW],
                            start=(j == 0), stop=(j == NP - 1),
                            perf_mode=DR, skip_group_check=True,
                        )

            dchain_dbg = {}

            def emit_dchain(kb):
                # q = clamped psum-scale p_joint: p*ln p folds ZPJ in later
                pclc = pool_d.tile([P, C1], BF16, name=f"pclc{kb}", tag="pclc")
                nc.vector.tensor_scalar(
                    pclc[:], pja[kb][:], EPSQ, None, op0=ALU.max, op1=ALU.add,
                    accum_out=b_clm[:, kb : kb + 1],
                )
                lp = pool_d.tile([P, C1], BF16, name=f"lp{kb}", tag="lp")
                nc.scalar.activation(lp[:], pclc[:], AF.Ln)
                plp = pool_d.tile([P, C1], BF16, name=f"plp{kb}", tag="plp")
                nc.vector.tensor_tensor(plp[:], lp[:], pclc[:], op=ALU.mult)
                nc.vector.reduce_sum(a_col[:, kb : kb + 1], plp[:], axis=AX.X)
                if kb == 0:
                    dchain_dbg.update(pclc=pclc, lp=lp, plp=plp)

            # ---------------- emission timeline ----------------
            # DMA order: 3:1 xu-favored so both su chunks (pairs 0-7, 8-15)
            # exchange early; xs pairs trail and pace the sweeps.
            dma_plan = []
            xu_iter = iter(range(NP))
            xs_iter = iter(range(NP))
            done_xu = 0
            while done_xu < NP:
                for _ in range(3):
                    j = next(xu_iter, None)
                    if j is not None:
                        dma_plan.append(("xu", j))
                        done_xu += 1
                b = next(xs_iter, None)
                if b is not None:
                    dma_plan.append(("xs", b))
            dma_plan += [("xs", b) for b in xs_iter]

            swept = 0
            for kind, j in dma_plan:
                if kind == "xu":
                    emit_xu(j)
                    if j == 7:
                        emit_su_chain(0)
                        for a in range(8):
                            emit_eu_apply(a)
                    elif j == NP - 1:
                        emit_su_chain(1)
                        for a in range(8, NP):
                            emit_eu_apply(a)
                else:
                    emit_xs(j)
                    if j == 4 * swept + 3:
                        emit_sweep(swept)
                        swept += 1
            while swept < NG:
                emit_sweep(swept)
                swept += 1

            # entropy chains per kb (gated on the group-3 flush of that kb)
            for kb in range(KB):
                emit_dchain(kb)

            # p_seen: scale+clamp+ln on [1,1024], redistribute to [128, kb]
            psv = stat.tile([1, C1], F32)
            for ch in range(CH):
                nc.vector.tensor_scalar(
                    psv[:, ch * CHW : (ch + 1) * CHW], ps_ps[ch][:],
                    ZPU, EPS, op0=ALU.mult, op1=ALU.max,
                )
            lpsv = stat.tile([1, C1], F32)
            nc.scalar.activation(lpsv[:], psv[:], AF.Ln)
            nc.sync.dma_start(
                lps_dram[:].rearrange("(a c) -> a c", a=1), lpsv[:]
            )
            lps = stat.tile([P, KB], F32)
            for kb in range(KB):
                nc.sync.dma_start(
                    lps[:, kb : kb + 1],
                    lps_dram[kb * P : (kb + 1) * P].rearrange("(p c) -> p c",
                                                              c=1),
                )
            lpsz = stat.tile([P, KB], F32)
            nc.vector.tensor_scalar(
                lpsz[:], lps[:], LNZ, None, op0=ALU.subtract
            )
            t2 = stat.tile([P, KB], F32)
            nc.vector.tensor_tensor(t2[:], lpsz[:], b_clm[:], op=ALU.mult)
            u = stat.tile([P, KB], F32)
            nc.vector.tensor_tensor(u[:], a_col[:], t2[:], op=ALU.subtract)
            s1c = stat.tile([P, 1], F32)
            nc.vector.reduce_sum(s1c[:], u[:], axis=AX.X)
            ps_fin = psum_fin.tile([1, 1], F32, tag="fin")
            nc.tensor.matmul(ps_fin[:], s1c[:], ones_f[:])

            # p_unseen entropy for the local half
            s2c = stat.tile([1, CH], F32)
            for ch in range(CH):
                puc = pool_v1.tile([1, CHW], F32, name=f"puc{ch}", tag="puc")
                nc.vector.tensor_scalar(
                    puc[:], pu_ps[ch][:], ZPU, EPS, op0=ALU.mult, op1=ALU.max
                )
                lpu = pool_v1.tile([1, CHW], F32, name=f"lpu{ch}", tag="lpu")
                nc.scalar.activation(lpu[:], puc[:], AF.Ln)
                pup = pool_v1.tile([1, CHW], F32, name=f"pup{ch}", tag="pup")
                nc.vector.tensor_tensor(pup[:], lpu[:], puc[:], op=ALU.mult)
                nc.vector.reduce_sum(s2c[:, ch : ch + 1], pup[:], axis=AX.X)
            s2 = stat.tile([1, 1], F32)
            nc.vector.reduce_sum(s2[:], s2c[:], axis=AX.X)

            fin = stat.tile([1, 2], F32)
            nc.scalar.mul(fin[:, 0:1], ps_fin[:], ZPJ)
            nc.vector.tensor_copy(fin[:, 1:2], s2[:])
            nc.sync.dma_start(out_d[:], fin[:])

            if debug:
                dbg = {
                    "d_su_tot": su_tot, "d_rsu": rsu, "d_ss": ss_all,
                    "d_b_clm": b_clm, "d_a_col": a_col, "d_lps": lps,
                    "d_psv": psv, "d_s2c": s2c,
                }
                for nm, t in dbg.items():
                    dd = nc.dram_tensor(nm, list(t.shape), F32,
                                        kind="ExternalOutput")
                    nc.sync.dma_start(dd[:], t[:])
                pj_d = nc.dram_tensor("d_pja0", [P, C1], F32,
                                      kind="ExternalOutput")
                nc.sync.dma_start(pj_d[:], pja[0][:])
                for nm, t in dchain_dbg.items():
                    dd = nc.dram_tensor(f"d_{nm}0", [P, C1], BF16,
                                        kind="ExternalOutput")
                    nc.sync.dma_start(dd[:], t[:])
                for nm, src_t in (("d_xsw0", xsw[0]), ("d_euq0", euq[0])):
                    deq = pool_xur.tile([P, 2, C1], F32, name=f"deq_{nm}",
                                        tag="xur")
                    nc.vector.tensor_copy(deq[:], src_t[:])
                    dd = nc.dram_tensor(nm, [P, 2 * C1], F32,
                                        kind="ExternalOutput")
                    nc.sync.dma_start(
                        dd[:], deq[:].rearrange("p i c -> p (i c)"))

    nc.finalize()
    return nc


def _get_nc():
    import os
    global _NC
    if _NC is None:
        _NC = _build_nc(debug=os.environ.get("KERNEL_DEBUG", "0") == "1")
    return _NC


def make_in_maps(x_seen, x_unseen):
    in_maps = []
    for h in range(H):
        for half in range(2):
            c0 = half * C1
            in_maps.append({
                "xu0": np.ascontiguousarray(x_unseen[0, h, :, c0 : c0 + C1]),
                "xu1": np.ascontiguousarray(x_unseen[1, h, :, c0 : c0 + C1]),
                "xs0": np.ascontiguousarray(x_seen[0, h]),
                "xs1": np.ascontiguousarray(x_seen[1, h]),
            })
    return in_maps


def kernel(x_seen: np.ndarray, x_unseen: np.ndarray) -> np.ndarray:
    import os

    global LAST_RESULTS
    nc = _get_nc()
    in_maps = make_in_maps(x_seen, x_unseen)
    trace = os.environ.get("KERNEL_TRACE", "0") == "1"
    kw = {}
    td = os.environ.get("KERNEL_TRACE_DIR")
    if td:
        kw["tmpdir"] = td
    res = run_bass_kernel_spmd(nc, in_maps, list(range(8)), trace=trace, **kw)
    LAST_RESULTS = res
    s1 = sum(float(r["parts"][0, 0]) for r in res.results)
    s2 = sum(float(r["parts"][0, 1]) for r in res.results)
    return np.array((s2 - s1) / H, dtype=np.float32)
